# revision 24
# baseline (speedup 1.0000x reference)
"""Causal self-attention (Q=K=V=x) Trainium2 Bass kernel.

Per-core program (data-parallel over batch, 1 batch element per core):
  scores = x @ x.T            -- fp16 hi/lo 3-pass matmul (fp32-grade accuracy),
                                 strictly-lower blocks only
  W = softmax(mask(scores))   -- chunked, deferred normalization:
                                 exp(s - m_chunk) on ScalarE with fused row-sum,
                                 then one per-chunk rescale by exp(m_c - m)/Z
  out = W @ x                 -- W blocks PE-transposed to fp16, fp16 matmul
Row 0 of the reference softmaxes an all -1e9 row -> uniform 1/T weights and
att_vec[0] = mean(x); both are patched at the end with a dedicated path.
"""

import os
import sys
from contextlib import ExitStack

for _p in ("/opt/trn_rl_repo", "/root/.axon_site/_ro/trn_rl_repo"):
    if os.path.isdir(_p):
        if _p not in sys.path:
            sys.path.insert(0, _p)
        break

import numpy as np

import concourse.mybir as mybir
import concourse.tile as tile
from concourse import bacc
from concourse.bass_utils import run_bass_kernel_spmd

F32 = mybir.dt.float32
F16 = mybir.dt.float16
X = mybir.AxisListType.X
Exp = mybir.ActivationFunctionType.Exp

B = 8
T = 2048
D = 1024
NEG_BIG = -2.0e9


def _chunks_for_block(I):
    # QK score chunks for t-block I: cover s in [0, (I+1)*128).
    # Full 512-wide chunks, then a tail of width 128*(I%4)+128 whose last
    # 128 columns are the diagonal block (causal mask applied there).
    n_full = I // 4
    chunks = [(k * 512, 512) for k in range(n_full)]
    tail_off = n_full * 512
    chunks.append((tail_off, (I + 1) * 128 - tail_off))
    return chunks


def _build(T_, D_, tmode="whole", zeros_dma=False):
    NT = T_ // 128  # t-blocks
    ND = D_ // 128  # contraction d-chunks
    NDC = D_ // 512  # output d-chunks

    nc = bacc.Bacc("TRN2", target_bir_lowering=False, debug=False)
    if tmode == "host":
        # host pre-splits x into fp16 hi (natural layout; lo is only ever
        # consumed transposed) and the blocked hi|lo transpose
        xhn_d = nc.dram_tensor("xhn", [T_, D_], F16, kind="ExternalInput").ap()
        xt_d = nc.dram_tensor(
            "xt", [T_ // 128, 128, 2 * (D_ // 128), 128], F16, kind="ExternalInput"
        ).ap()
    else:
        x_d = nc.dram_tensor("x", [T_, D_], F32, kind="ExternalInput").ap()
    av_d = nc.dram_tensor("att_vec", [T_, D_], F32, kind="ExternalOutput").ap()
    aw_d = nc.dram_tensor("att_w", [T_, T_], F32, kind="ExternalOutput").ap()


    with tile.TileContext(nc) as tc, ExitStack() as ctx:
        singles = ctx.enter_context(tc.tile_pool(name="singles", bufs=1))
        xres = ctx.enter_context(tc.tile_pool(name="xres", bufs=1))
        stage = ctx.enter_context(tc.tile_pool(name="stage", bufs=4))
        wpool = ctx.enter_context(tc.tile_pool(name="wpool", bufs=2))
        wtpool = ctx.enter_context(tc.tile_pool(name="wtpool", bufs=2))
        avpool = ctx.enter_context(tc.tile_pool(name="avpool", bufs=3))
        stats = ctx.enter_context(tc.tile_pool(name="stats", bufs=2))
        qk_ps = ctx.enter_context(tc.tile_pool(name="qk_ps", bufs=6, space="PSUM"))
        av_ps = ctx.enter_context(tc.tile_pool(name="av_ps", bufs=2, space="PSUM"))

        # constants
        maskt = singles.tile([128, 128], F32)
        nc.vector.memset(maskt[:], 0.0)
        # keep 0 where s < t (strictly lower), NEG_BIG on diagonal and above
        nc.gpsimd.affine_select(
            out=maskt[:],
            in_=maskt[:],
            compare_op=mybir.AluOpType.is_ge,
            fill=NEG_BIG,
            base=-1,
            pattern=[[-1, 128]],
            channel_multiplier=1,
        )
        if zeros_dma:
            zeros = singles.tile([128, T_ - 128], F32)
            nc.vector.memset(zeros[:], 0.0)
        ones_h = singles.tile([128, 1], F16)
        nc.vector.memset(ones_h[:], 1.0)

        # load x, split fp32 -> fp16 hi (resident) + fp16 lo; build x^T for
        # both via 16-bit xbar DMA transposes (strategy set by tmode)
        # resident fp16 hi x in natural layout (AV matmul rhs); in host
        # mode the lo part lives only in the transposed tensor
        nat_w = D_ if tmode == "host" else 2 * D_
        xh_all = xres.tile([128, NT, nat_w], F16)
        xh = xh_all[:, :, 0:D_]
        # combined transpose target; host mode uses a t-block-major layout
        # [d_part, t_block, chunk, t_within] so input loads are contiguous
        if tmode == "host":
            xt_blk = xres.tile([128, NT, 2 * ND, 128], F16)
            xth = xtl = None
        else:
            xt_all = xres.tile([128, 2 * ND, T_], F16)
            xth = xt_all[:, 0:ND, :]  # x.T hi: [d_part, d_chunk, t]
            xtl = xt_all[:, ND : 2 * ND, :]
        if tmode in ("whole", "half", "quarter"):
            ngrp = {"whole": 1, "half": 2, "quarter": 4}[tmode]
            rows_g = T_ // ngrp
            blk_g = rows_g // 128
            xh_ds = [
                nc.dram_tensor(f"xh_scr{q}", [rows_g, D_], F16).ap()
                for q in range(ngrp)
            ]
            xl_ds = [
                nc.dram_tensor(f"xl_scr{q}", [rows_g, D_], F16).ap()
                for q in range(ngrp)
            ]
        for i in range(NT):
            r = slice(i * 128, (i + 1) * 128)
            if tmode == "host":
                nc.sync.dma_start(xt_blk[:, i, :, :], xt_d[i])
                nc.sync.dma_start(xh_all[:, i, :], xhn_d[r, :])
                continue
            xs = stage.tile([128, D_], F32, tag="xs")
            nc.sync.dma_start(xs[:], x_d[r, :])
            if tmode == "sbuf1":
                # cast on the idle GpSimd engine, lo-residual on VectorE; one
                # blocked-transpose op per block covers both hi and lo
                nc.gpsimd.tensor_copy(xh_all[:, i, 0:D_], xs[:])
                nc.vector.tensor_tensor(
                    xh_all[:, i, D_ : 2 * D_],
                    xs[:],
                    xh_all[:, i, 0:D_],
                    op=mybir.AluOpType.subtract,
                )
                nc.sync.dma_start_transpose(xt_all[:, :, r], xh_all[:, i, :])
                continue
            nc.vector.tensor_copy(xh[:, i, :], xs[:])
            xl = stage.tile([128, D_], F16, tag="xl")
            nc.vector.tensor_tensor(
                xl[:], xs[:], xh[:, i, :], op=mybir.AluOpType.subtract
            )
            if tmode == "sbuf":
                for j in range(ND):
                    c = slice(j * 128, (j + 1) * 128)
                    nc.sync.dma_start_transpose(xth[:, j, r], xh[:, i, c])
                    nc.sync.dma_start_transpose(xtl[:, j, r], xl[:, c])
            else:
                q, qr = divmod(i, blk_g)
                qs = slice(qr * 128, (qr + 1) * 128)
                nc.sync.dma_start(xh_ds[q][qs, :], xh[:, i, :])
                nc.sync.dma_start(xl_ds[q][qs, :], xl[:])
                if qr == blk_g - 1:
                    tr = slice(q * rows_g, (q + 1) * rows_g)
                    for j in range(ND):
                        c = slice(j * 128, (j + 1) * 128)
                        nc.sync.dma_start_transpose(xth[:, j, tr], xh_ds[q][:, c])
                        nc.sync.dma_start_transpose(xtl[:, j, tr], xl_ds[q][:, c])

        def emit_qk_softmax(I):
            chunks = _chunks_for_block(I)
            nch = len(chunks)
            wrow = wpool.tile([128, T_], F32, tag="wrow")
            negm = stats.tile([128, 4], F32, tag="negm")
            zc = stats.tile([128, 4], F32, tag="zc")
            t0 = I * 128
            if tmode == "host":
                # weights-outer / chunk-inner: one LDWEIGHTS serves all chunks
                pqks = [
                    qk_ps.tile([128, 512], F32, tag="pqk", name=f"pqk{c}")
                    for c in range(nch)
                ]
                passes_h = ((0, 0), (0, ND), (ND, 0))
                for pi, (ja, jb) in enumerate(passes_h):
                    for dj in range(ND):
                        for c, (off, w) in enumerate(chunks):
                            bs, nb = off // 128, w // 128
                            nc.tensor.matmul(
                                pqks[c][:, :w],
                                xt_blk[:, I, ja + dj, :],
                                xt_blk[:, bs : bs + nb, jb + dj, :],
                                start=(pi == 0 and dj == 0),
                                stop=(pi == 2 and dj == ND - 1),
                            )
            for c, (off, w) in enumerate(chunks):
                if tmode == "host":
                    pqk = pqks[c]
                else:
                    pqk = qk_ps.tile([128, 512], F32, tag="pqk")
                    passes = ((xth, xth), (xth, xtl), (xtl, xth))
                    for pi, (A, Bm) in enumerate(passes):
                        for dj in range(ND):
                            nc.tensor.matmul(
                                pqk[:, :w],
                                A[:, dj, t0 : t0 + 128],
                                Bm[:, dj, off : off + w],
                                start=(pi == 0 and dj == 0),
                                stop=(pi == 2 and dj == ND - 1),
                            )
                if c == nch - 1:  # tail chunk ends with the diagonal block
                    nc.vector.tensor_tensor(
                        pqk[:, w - 128 : w],
                        pqk[:, w - 128 : w],
                        maskt[:],
                        op=mybir.AluOpType.add,
                    )
                nc.vector.tensor_reduce(
                    negm[:, c : c + 1],
                    pqk[:, :w],
                    axis=X,
                    op=mybir.AluOpType.max,
                    negate=True,
                )
                nc.scalar.activation(
                    out=wrow[:, off : off + w],
                    in_=pqk[:, :w],
                    func=Exp,
                    bias=negm[:, c : c + 1],
                    scale=1.0,
                    accum_out=zc[:, c : c + 1],
                )
            # combine chunk stats: m = max_c m_c (negm holds -m_c)
            negm_t = stats.tile([128, 1], F32, tag="negm_t")
            nc.vector.tensor_reduce(
                negm_t[:], negm[:, :nch], axis=X, op=mybir.AluOpType.min
            )
            alpha = stats.tile([128, 4], F32, tag="alpha")  # exp(m_c - m)
            nc.scalar.activation(
                out=alpha[:, :nch],
                in_=negm[:, :nch],
                func=Exp,
                bias=negm_t[:, 0:1],
                scale=-1.0,
            )
            zs = stats.tile([128, 4], F32, tag="zs")
            nc.vector.tensor_tensor(
                zs[:, :nch], zc[:, :nch], alpha[:, :nch], op=mybir.AluOpType.mult
            )
            ztot = stats.tile([128, 1], F32, tag="ztot")
            nc.vector.tensor_reduce(
                ztot[:], zs[:, :nch], axis=X, op=mybir.AluOpType.add
            )
            rz = stats.tile([128, 1], F32, tag="rz")
            nc.vector.reciprocal(rz[:], ztot[:])
            rho = stats.tile([128, 4], F32, tag="rho")
            nc.vector.tensor_scalar_mul(rho[:, :nch], alpha[:, :nch], rz[:, 0:1])
            wrow16 = wpool.tile([128, T_], F16, tag="wrow16")
            for c, (off, w) in enumerate(chunks):
                nc.vector.tensor_scalar_mul(
                    wrow16[:, off : off + w], wrow[:, off : off + w], rho[:, c : c + 1]
                )
                nc.vector.tensor_scalar_mul(
                    wrow[:, off : off + w], wrow[:, off : off + w], rho[:, c : c + 1]
                )
            # store W row-block (row 0 of block 0 is patched separately; the
            # upper triangle relies on the runtime's zero-initialized outputs)
            r0 = 1 if I == 0 else 0
            wcols = (I + 1) * 128
            nc.sync.dma_start(
                aw_d[t0 + r0 : t0 + 128, 0:wcols], wrow[r0:, :wcols]
            )
            if zeros_dma and wcols < T_:
                nc.sync.dma_start(
                    aw_d[t0 + r0 : t0 + 128, wcols:T_], zeros[r0:, : T_ - wcols]
                )
            return wrow16

        def emit_transp_av(I, wrow16):
            wt = wtpool.tile([128, NT, 128], F16, tag="wt")
            nc.sync.dma_start_transpose(
                wt[:, 0 : I + 1, :], wrow16[:, 0 : (I + 1) * 128]
            )
            r0 = 1 if I == 0 else 0
            for dc in range(NDC):
                pav = av_ps.tile([128, 512], F32, tag="pav")
                for J in range(I + 1):
                    nc.tensor.matmul(
                        pav[:],
                        wt[:, J, :],
                        xh[:, J, dc * 512 : (dc + 1) * 512],
                        start=(J == 0),
                        stop=(J == I),
                    )
                av_sb = avpool.tile([128, 512], F32, tag="av_sb")
                nc.scalar.copy(av_sb[:], pav[:])
                nc.sync.dma_start(
                    av_d[I * 128 + r0 : (I + 1) * 128, dc * 512 : (dc + 1) * 512],
                    av_sb[r0:, :],
                )

        # pipeline: transpose+AV of block I-1 is emitted after QK of block I so
        # the PE never stalls waiting on block I-1's softmax
        pending = None
        for I in range(NT):
            wrow = emit_qk_softmax(I)
            if pending is not None:
                emit_transp_av(*pending)
            pending = (I, wrow)
        emit_transp_av(*pending)

        # row 0: att_vec[0] = mean(x), att_w[0, :] = 1/T
        mean_sb = avpool.tile([1, D_], F32, tag="mean_sb")
        for dc in range(NDC):
            pmean = av_ps.tile([1, 512], F32, tag="pav")
            for J in range(NT):
                nc.tensor.matmul(
                    pmean[:],
                    ones_h[:, 0:1],
                    xh[:, J, dc * 512 : (dc + 1) * 512],
                    start=(J == 0),
                    stop=(J == NT - 1),
                )
            nc.scalar.mul(mean_sb[:, dc * 512 : (dc + 1) * 512], pmean[:], 1.0 / T_)
        nc.sync.dma_start(av_d[0:1, :], mean_sb[:])
        w0 = singles.tile([1, T_], F32)
        nc.vector.memset(w0[:], 1.0 / T_)
        nc.sync.dma_start(aw_d[0:1, :], w0[:])

    nc.compile()
    return nc


_NC_CACHE = {}


def _get_nc(T_, D_):
    key = (T_, D_)
    if key not in _NC_CACHE:
        _NC_CACHE[key] = _build(T_, D_, tmode="host")
    return _NC_CACHE[key]


def _prep_inputs(xb):
    # split into fp16 hi + lo (x ~= hi + lo to ~22 mantissa bits) and build
    # the [d_part, chunk, t] blocked transpose the QK matmuls consume
    h = xb.astype(np.float16)
    l = (xb - h.astype(np.float32)).astype(np.float16)
    hl = np.concatenate([h, l], axis=1)  # [T, 2D]
    T_ = xb.shape[0]
    # xt[i, p, j, tw] = hl[i*128 + tw, j*128 + p]: per-t-block contiguous
    # slab, matching the on-device [d_part, t_block, chunk, t_within] layout
    xt = np.ascontiguousarray(
        np.transpose(hl.reshape(T_ // 128, 128, -1, 128), (0, 3, 2, 1))
    )
    return {"xhn": h, "xt": xt}


def kernel(x: np.ndarray):
    assert x.shape == (B, T, D), x.shape
    nc = _get_nc(T, D)
    in_maps = [_prep_inputs(np.asarray(x[b], dtype=np.float32)) for b in range(B)]
    res = run_bass_kernel_spmd(nc, in_maps, list(range(B))).results
    att_vec = np.stack([res[b]["att_vec"] for b in range(B)])
    att_w = np.stack([res[b]["att_w"] for b in range(B)])
    return att_vec, att_w


# revision 33
# speedup vs baseline: 1.0256x; 1.0256x over previous
"""Causal self-attention (Q=K=V=x) Trainium2 Bass kernel.

Per-core program (data-parallel over batch, 1 batch element per core):
  scores = x @ x.T            -- fp16 hi/lo 3-pass matmul (fp32-grade accuracy),
                                 strictly-lower blocks only
  W = softmax(mask(scores))   -- chunked, deferred normalization:
                                 exp(s - m_chunk) on ScalarE with fused row-sum,
                                 then one per-chunk rescale by exp(m_c - m)/Z
  out = W @ x                 -- W blocks PE-transposed to fp16, fp16 matmul
Row 0 of the reference softmaxes an all -1e9 row -> uniform 1/T weights and
att_vec[0] = mean(x); both are patched at the end with a dedicated path.
"""

import os
import sys
from contextlib import ExitStack

for _p in ("/opt/trn_rl_repo", "/root/.axon_site/_ro/trn_rl_repo"):
    if os.path.isdir(_p):
        if _p not in sys.path:
            sys.path.insert(0, _p)
        break

import numpy as np

import concourse.mybir as mybir
import concourse.tile as tile
from concourse import bacc, bass_isa
from concourse.bass_utils import run_bass_kernel_spmd

F32 = mybir.dt.float32
F16 = mybir.dt.float16
X = mybir.AxisListType.X
Exp = mybir.ActivationFunctionType.Exp

B = 8
T = 2048
D = 1024
NEG_BIG = -2.0e9


def _chunks_for_block(I):
    # QK score chunks for t-block I: cover s in [0, (I+1)*128).
    # Full 512-wide chunks, then a tail of width 128*(I%4)+128 whose last
    # 128 columns are the diagonal block (causal mask applied there).
    n_full = I // 4
    chunks = [(k * 512, 512) for k in range(n_full)]
    tail_off = n_full * 512
    chunks.append((tail_off, (I + 1) * 128 - tail_off))
    return chunks


def _build(T_, D_, tmode="whole", zeros_dma=False):
    NT = T_ // 128  # t-blocks
    ND = D_ // 128  # contraction d-chunks
    NDC = D_ // 512  # output d-chunks

    nc = bacc.Bacc("TRN2", target_bir_lowering=False, debug=False)
    if tmode == "host":
        # host pre-splits x into fp16 hi (natural layout; lo is only ever
        # consumed transposed) and the blocked hi|lo transpose
        xhn_d = nc.dram_tensor("xhn", [T_, D_], F16, kind="ExternalInput").ap()
        xt_d = nc.dram_tensor(
            "xt", [T_ // 128, 128, 2 * (D_ // 128), 128], F16, kind="ExternalInput"
        ).ap()
    else:
        x_d = nc.dram_tensor("x", [T_, D_], F32, kind="ExternalInput").ap()
    av_d = nc.dram_tensor("att_vec", [T_, D_], F32, kind="ExternalOutput").ap()
    aw_d = nc.dram_tensor("att_w", [T_, T_], F32, kind="ExternalOutput").ap()


    with tile.TileContext(nc) as tc, ExitStack() as ctx:
        singles = ctx.enter_context(tc.tile_pool(name="singles", bufs=1))
        xres = ctx.enter_context(tc.tile_pool(name="xres", bufs=1))
        stage = ctx.enter_context(tc.tile_pool(name="stage", bufs=4))
        wpool = ctx.enter_context(tc.tile_pool(name="wpool", bufs=2))
        wtpool = ctx.enter_context(tc.tile_pool(name="wtpool", bufs=2))
        avpool = ctx.enter_context(tc.tile_pool(name="avpool", bufs=3))
        stats = ctx.enter_context(tc.tile_pool(name="stats", bufs=2))
        qk_ps = ctx.enter_context(tc.tile_pool(name="qk_ps", bufs=6, space="PSUM"))
        av_ps = ctx.enter_context(tc.tile_pool(name="av_ps", bufs=2, space="PSUM"))

        # constants
        maskt = singles.tile([128, 128], F32)
        nc.vector.memset(maskt[:], 0.0)
        # keep 0 where s < t (strictly lower), NEG_BIG on diagonal and above
        nc.gpsimd.affine_select(
            out=maskt[:],
            in_=maskt[:],
            compare_op=mybir.AluOpType.is_ge,
            fill=NEG_BIG,
            base=-1,
            pattern=[[-1, 128]],
            channel_multiplier=1,
        )
        if zeros_dma:
            zeros = singles.tile([128, T_ - 128], F32)
            nc.vector.memset(zeros[:], 0.0)
        ones_h = singles.tile([128, 1], F16)
        nc.vector.memset(ones_h[:], 1.0)

        # load x, split fp32 -> fp16 hi (resident) + fp16 lo; build x^T for
        # both via 16-bit xbar DMA transposes (strategy set by tmode)
        # resident fp16 hi x in natural layout (AV matmul rhs); in host
        # mode the lo part lives only in the transposed tensor
        nat_w = D_ if tmode == "host" else 2 * D_
        xh_all = xres.tile([128, NT, nat_w], F16)
        xh = xh_all[:, :, 0:D_]
        # combined transpose target; host mode uses a t-block-major layout
        # [d_part, t_block, chunk, t_within] so input loads are contiguous
        if tmode == "host":
            xt_blk = xres.tile([128, NT, 2 * ND, 128], F16)
            xth = xtl = None
        else:
            xt_all = xres.tile([128, 2 * ND, T_], F16)
            xth = xt_all[:, 0:ND, :]  # x.T hi: [d_part, d_chunk, t]
            xtl = xt_all[:, ND : 2 * ND, :]
        if tmode in ("whole", "half", "quarter"):
            ngrp = {"whole": 1, "half": 2, "quarter": 4}[tmode]
            rows_g = T_ // ngrp
            blk_g = rows_g // 128
            xh_ds = [
                nc.dram_tensor(f"xh_scr{q}", [rows_g, D_], F16).ap()
                for q in range(ngrp)
            ]
            xl_ds = [
                nc.dram_tensor(f"xl_scr{q}", [rows_g, D_], F16).ap()
                for q in range(ngrp)
            ]
        for i in range(NT):
            r = slice(i * 128, (i + 1) * 128)
            if tmode == "host":
                nc.sync.dma_start(xt_blk[:, i, 0:ND, :], xt_d[i, :, 0:ND, :])
                nc.sync.dma_start(
                    xt_blk[:, i, ND : 2 * ND, :], xt_d[i, :, ND : 2 * ND, :]
                )
                nc.sync.dma_start(xh_all[:, i, :], xhn_d[r, :])
                continue
            xs = stage.tile([128, D_], F32, tag="xs")
            nc.sync.dma_start(xs[:], x_d[r, :])
            if tmode == "sbuf1":
                # cast on the idle GpSimd engine, lo-residual on VectorE; one
                # blocked-transpose op per block covers both hi and lo
                nc.gpsimd.tensor_copy(xh_all[:, i, 0:D_], xs[:])
                nc.vector.tensor_tensor(
                    xh_all[:, i, D_ : 2 * D_],
                    xs[:],
                    xh_all[:, i, 0:D_],
                    op=mybir.AluOpType.subtract,
                )
                nc.sync.dma_start_transpose(xt_all[:, :, r], xh_all[:, i, :])
                continue
            nc.vector.tensor_copy(xh[:, i, :], xs[:])
            xl = stage.tile([128, D_], F16, tag="xl")
            nc.vector.tensor_tensor(
                xl[:], xs[:], xh[:, i, :], op=mybir.AluOpType.subtract
            )
            if tmode == "sbuf":
                for j in range(ND):
                    c = slice(j * 128, (j + 1) * 128)
                    nc.sync.dma_start_transpose(xth[:, j, r], xh[:, i, c])
                    nc.sync.dma_start_transpose(xtl[:, j, r], xl[:, c])
            else:
                q, qr = divmod(i, blk_g)
                qs = slice(qr * 128, (qr + 1) * 128)
                nc.sync.dma_start(xh_ds[q][qs, :], xh[:, i, :])
                nc.sync.dma_start(xl_ds[q][qs, :], xl[:])
                if qr == blk_g - 1:
                    tr = slice(q * rows_g, (q + 1) * rows_g)
                    for j in range(ND):
                        c = slice(j * 128, (j + 1) * 128)
                        nc.sync.dma_start_transpose(xth[:, j, tr], xh_ds[q][:, c])
                        nc.sync.dma_start_transpose(xtl[:, j, tr], xl_ds[q][:, c])

        # att_vec row 0 = mean(x): block-sum chain on the otherwise idle
        # GpSimd engine (pipelines with the input loads), then a
        # cross-partition all-reduce, also on GpSimd
        msum = singles.tile([128, D_], F32)
        nc.gpsimd.tensor_tensor(
            msum[:], xh[:, 0, :], xh[:, 1, :], op=mybir.AluOpType.add
        )
        for i in range(2, NT):
            nc.gpsimd.tensor_tensor(
                msum[:], msum[:], xh[:, i, :], op=mybir.AluOpType.add
            )
        mall = singles.tile([128, D_], F32)
        nc.gpsimd.partition_all_reduce(
            mall[:], msum[:], channels=128, reduce_op=bass_isa.ReduceOp.add
        )
        mean_sb = avpool.tile([1, D_], F32, tag="mean_sb")
        nc.scalar.mul(mean_sb[:], mall[0:1, :], 1.0 / T_)

        def emit_qk_softmax(I):
            chunks = _chunks_for_block(I)
            nch = len(chunks)
            wrow = wpool.tile([128, T_], F32, tag="wrow")
            negm = stats.tile([128, 4], F32, tag="negm")
            zc = stats.tile([128, 4], F32, tag="zc")
            t0 = I * 128
            if tmode == "host":
                # weights-outer / chunk-inner: one LDWEIGHTS serves all chunks
                pqks = [
                    qk_ps.tile([128, 512], F32, tag="pqk", name=f"pqk{c}")
                    for c in range(nch)
                ]
                passes_h = ((0, 0), (0, ND), (ND, 0))
                for pi, (ja, jb) in enumerate(passes_h):
                    for dj in range(ND):
                        for c, (off, w) in enumerate(chunks):
                            bs, nb = off // 128, w // 128
                            nc.tensor.matmul(
                                pqks[c][:, :w],
                                xt_blk[:, I, ja + dj, :],
                                xt_blk[:, bs : bs + nb, jb + dj, :],
                                start=(pi == 0 and dj == 0),
                                stop=(pi == 2 and dj == ND - 1),
                            )
            for c, (off, w) in enumerate(chunks):
                if tmode == "host":
                    pqk = pqks[c]
                else:
                    pqk = qk_ps.tile([128, 512], F32, tag="pqk")
                    passes = ((xth, xth), (xth, xtl), (xtl, xth))
                    for pi, (A, Bm) in enumerate(passes):
                        for dj in range(ND):
                            nc.tensor.matmul(
                                pqk[:, :w],
                                A[:, dj, t0 : t0 + 128],
                                Bm[:, dj, off : off + w],
                                start=(pi == 0 and dj == 0),
                                stop=(pi == 2 and dj == ND - 1),
                            )
                if c == nch - 1:  # tail chunk ends with the diagonal block
                    nc.vector.tensor_tensor(
                        pqk[:, w - 128 : w],
                        pqk[:, w - 128 : w],
                        maskt[:],
                        op=mybir.AluOpType.add,
                    )
                nc.vector.tensor_reduce(
                    negm[:, c : c + 1],
                    pqk[:, :w],
                    axis=X,
                    op=mybir.AluOpType.max,
                    negate=True,
                )
                nc.scalar.activation(
                    out=wrow[:, off : off + w],
                    in_=pqk[:, :w],
                    func=Exp,
                    bias=negm[:, c : c + 1],
                    scale=1.0,
                    accum_out=zc[:, c : c + 1],
                )
            # combine chunk stats: m = max_c m_c (negm holds -m_c)
            negm_t = stats.tile([128, 1], F32, tag="negm_t")
            nc.vector.tensor_reduce(
                negm_t[:], negm[:, :nch], axis=X, op=mybir.AluOpType.min
            )
            alpha = stats.tile([128, 4], F32, tag="alpha")  # exp(m_c - m)
            nc.scalar.activation(
                out=alpha[:, :nch],
                in_=negm[:, :nch],
                func=Exp,
                bias=negm_t[:, 0:1],
                scale=-1.0,
            )
            zs = stats.tile([128, 4], F32, tag="zs")
            nc.vector.tensor_tensor(
                zs[:, :nch], zc[:, :nch], alpha[:, :nch], op=mybir.AluOpType.mult
            )
            ztot = stats.tile([128, 1], F32, tag="ztot")
            nc.vector.tensor_reduce(
                ztot[:], zs[:, :nch], axis=X, op=mybir.AluOpType.add
            )
            rz = stats.tile([128, 1], F32, tag="rz")
            nc.vector.reciprocal(rz[:], ztot[:])
            rho = stats.tile([128, 4], F32, tag="rho")
            nc.vector.tensor_scalar_mul(rho[:, :nch], alpha[:, :nch], rz[:, 0:1])
            wrow16 = wpool.tile([128, T_], F16, tag="wrow16")
            for c, (off, w) in enumerate(chunks):
                nc.vector.tensor_scalar_mul(
                    wrow16[:, off : off + w], wrow[:, off : off + w], rho[:, c : c + 1]
                )
                nc.vector.tensor_scalar_mul(
                    wrow[:, off : off + w], wrow[:, off : off + w], rho[:, c : c + 1]
                )
            # store W row-block (row 0 of block 0 is patched separately; the
            # upper triangle relies on the runtime's zero-initialized outputs)
            r0 = 1 if I == 0 else 0
            wcols = (I + 1) * 128
            nc.sync.dma_start(
                aw_d[t0 + r0 : t0 + 128, 0:wcols], wrow[r0:, :wcols]
            )
            if zeros_dma and wcols < T_:
                nc.sync.dma_start(
                    aw_d[t0 + r0 : t0 + 128, wcols:T_], zeros[r0:, : T_ - wcols]
                )
            return wrow16

        def emit_transp_av(I, wrow16):
            wt = wtpool.tile([128, NT, 128], F16, tag="wt")
            nc.sync.dma_start_transpose(
                wt[:, 0 : I + 1, :], wrow16[:, 0 : (I + 1) * 128]
            )
            r0 = 1 if I == 0 else 0
            for dc in range(NDC):
                pav = av_ps.tile([128, 512], F32, tag="pav")
                for J in range(I + 1):
                    nc.tensor.matmul(
                        pav[:],
                        wt[:, J, :],
                        xh[:, J, dc * 512 : (dc + 1) * 512],
                        start=(J == 0),
                        stop=(J == I),
                    )
                av_sb = avpool.tile([128, 512], F32, tag="av_sb")
                nc.scalar.copy(av_sb[:], pav[:])
                nc.sync.dma_start(
                    av_d[I * 128 + r0 : (I + 1) * 128, dc * 512 : (dc + 1) * 512],
                    av_sb[r0:, :],
                )

        # pipeline: transpose+AV of block I-1 is emitted after QK of block I so
        # the PE never stalls waiting on block I-1's softmax
        pending = None
        for I in range(NT):
            wrow = emit_qk_softmax(I)
            if pending is not None:
                emit_transp_av(*pending)
            pending = (I, wrow)
        emit_transp_av(*pending)

        # row 0: att_vec[0] = mean(x) (emitted earlier, on GpSimd)
        nc.sync.dma_start(av_d[0:1, :], mean_sb[:])
        w0 = singles.tile([1, T_], F32)
        nc.vector.memset(w0[:], 1.0 / T_)
        nc.sync.dma_start(aw_d[0:1, :], w0[:])

    nc.compile()
    return nc


_NC_CACHE = {}


def _get_nc(T_, D_):
    key = (T_, D_)
    if key not in _NC_CACHE:
        _NC_CACHE[key] = _build(T_, D_, tmode="host")
    return _NC_CACHE[key]


def _prep_inputs(xb):
    # split into fp16 hi + lo (x ~= hi + lo to ~22 mantissa bits) and build
    # the [d_part, chunk, t] blocked transpose the QK matmuls consume
    h = xb.astype(np.float16)
    l = (xb - h.astype(np.float32)).astype(np.float16)
    hl = np.concatenate([h, l], axis=1)  # [T, 2D]
    T_ = xb.shape[0]
    # xt[i, p, j, tw] = hl[i*128 + tw, j*128 + p]: per-t-block contiguous
    # slab, matching the on-device [d_part, t_block, chunk, t_within] layout
    xt = np.ascontiguousarray(
        np.transpose(hl.reshape(T_ // 128, 128, -1, 128), (0, 3, 2, 1))
    )
    return {"xhn": h, "xt": xt}


def kernel(x: np.ndarray):
    assert x.shape == (B, T, D), x.shape
    nc = _get_nc(T, D)
    in_maps = [_prep_inputs(np.asarray(x[b], dtype=np.float32)) for b in range(B)]
    res = run_bass_kernel_spmd(nc, in_maps, list(range(B))).results
    att_vec = np.stack([res[b]["att_vec"] for b in range(B)])
    att_w = np.stack([res[b]["att_w"] for b in range(B)])
    return att_vec, att_w
